# revision 29
# baseline (speedup 1.0000x reference)
"""GQA causal-attention prefill kernel for Trainium2, tensor-parallel over 8
NeuronCores.  v3: fp16 operands, static PSUM banks, interleaved schedule,
matmul-free softmax denominator.

Reference semantics: q/k/v projections + RoPE + causal GQA attention + output
projection, fp32, B=2, T=2048, D=4096, 32 q heads, 8 kv heads, head_dim 128.

Sharding: head-parallel. Core c gets q heads [4c, 4c+4), kv head c, and the
matching wo slice; each core computes a full-shape partial output
o_part = attn(heads of c) @ wo_c (fp16) and the host sums the 8 partials in
fp32.

Design (v1 baseline 1.29ms -> v2 1.15ms -> v3):
  - All matmul operands fp16: full PE rate at any free width, DVE 2-byte 2x
    mode, half DMA, no f32r cast penalties. PSUM f32. rel err ~5e-4.
  - Single TileContext-lifetime pools: SBUF addresses never migrate between
    phases (v1 lost 22us/batch to cross-phase SBUF reuse stalls).
  - PSUM banks statically time-shared:
      b01,b23 ([128,1024] = 2 banks each): proj q0..q3 halves, then the
        o-projection rotates its accumulation groups over the four 512-wide
        halves (4-deep rotation hides the evictions).
      b4,b5: proj k,v, then av accumulation (alternating heads)
      b6,b7: scores double-buffer (+ v-transpose staging on b7)
  - Interleaved schedule: proj(chunk c) -> attention(q-chunk c-1) with the
    o-projection of the block before that woven in per head. The Tile
    scheduler then overlaps everything; the PE stream has no phase cliffs
    and stays in the 2.4GHz p-state (idle gaps drop it to 1.2GHz).
  - softmax denominator without matmuls: DVE keeps a running fp16 sum of the
    exp'd score blocks (2x mode, off the critical path), GpSimd
    partition_all_reduce turns it into the broadcast denominator, DVE
    reciprocal + multiply fold 1/l into the attention output. This removes
    320 PE matmuls (~98us) the v1/v2 kernels spent on ones-column matmuls.
  - exp on ACT writes pT2 fp16 directly for off-diagonal blocks; diagonal
    blocks get a 0/1 wedge-mask multiply on DVE.
  - o-projection evictions alternate ACT/DVE so neither engine's queue gates
    the PE's o-matmul bank rotation.
"""

import os
import sys

sys.path.insert(0, "/opt/trn_rl_repo")

import numpy as np

B = 2
T = 2048
TOK = B * T
D = 4096
NQ = 32
NKV = 8
H = 128
HH = H // 2
THETA = 10000.0
NCORES = 8
NHC = NQ // NCORES          # q heads per core (4)
KPC = D // H                # contraction chunks of 128 over D (32)
TCH = 512                   # token chunk
NTCH = T // TCH             # 4 token chunks per batch
NSUB = TCH // H             # 4 128-sub-tiles per chunk
C_SM = 1.0 / np.sqrt(H)     # softmax scale
DEPTH = 3                   # score->av software pipeline depth


def _build_bass():
    import concourse.bacc as bacc
    import concourse.mybir as mybir
    import concourse.tile as tile
    from concourse import bass_isa
    from concourse.masks import make_identity
    from contextlib import ExitStack

    f16 = mybir.dt.float16
    f32 = mybir.dt.float32
    Exp = mybir.ActivationFunctionType.Exp
    Copy = mybir.ActivationFunctionType.Copy

    nc = bacc.Bacc("TRN2", target_bir_lowering=False, debug=False,
                   num_devices=NCORES)

    xT = nc.declare_dram_parameter("xT", [D, TOK], f16, isOutput=False)
    wq = nc.declare_dram_parameter("wq", [NHC, D, H], f16, isOutput=False)
    wk = nc.declare_dram_parameter("wk", [D, H], f16, isOutput=False)
    wv = nc.declare_dram_parameter("wv", [D, H], f16, isOutput=False)
    wo = nc.declare_dram_parameter("wo", [NHC, H, D], f16, isOutput=False)
    # rope tables with both partition halves duplicated (row p == row p+64)
    cosT = nc.declare_dram_parameter("cosT", [H, TOK], f16, isOutput=False)
    sinT = nc.declare_dram_parameter("sinT", [H, TOK], f16, isOutput=False)
    o_part = nc.declare_dram_parameter("o_part", [TOK, D], f16, isOutput=True)

    with tile.TileContext(nc) as tc:
        with ExitStack() as top:
            consts = top.enter_context(tc.tile_pool(name="consts", bufs=1))
            wpool = top.enter_context(tc.tile_pool(name="wpool", bufs=1))
            acts = top.enter_context(tc.tile_pool(name="acts", bufs=1))
            xpool = top.enter_context(tc.tile_pool(name="xpool", bufs=1))
            rope = top.enter_context(tc.tile_pool(name="rope", bufs=1))
            ppool = top.enter_context(tc.tile_pool(name="ppool", bufs=1))
            small = top.enter_context(tc.tile_pool(name="small", bufs=1))
            otpool = top.enter_context(tc.tile_pool(name="otpool", bufs=1))
            opool = top.enter_context(tc.tile_pool(name="opool", bufs=1))
            pbank = top.enter_context(
                tc.tile_pool(name="pbank", bufs=1, space="PSUM"))

            # ---- constants ----
            # fp16 identity: stationary operand for the additive-mask matmul
            identity = consts.tile([H, H], f16)
            make_identity(nc, identity)
            # additive causal wedge masks: 0 where t - s - 128*j >= 0
            # (allowed), -30000 where masked; accumulated into the scores
            # PSUM via I.T @ maskneg so the causal mask costs one PE matmul
            # instead of a DVE multiply on the exp->av critical path.
            masks = []
            for j in range(NSUB):
                m = consts.tile([H, TCH], f16, tag=f"mask{j}",
                                name=f"mask{j}")
                nc.vector.memset(m, 0.0)
                nc.gpsimd.affine_select(
                    out=m, in_=m,
                    compare_op=mybir.AluOpType.is_ge,
                    fill=-30000.0,
                    base=-H * j,
                    pattern=[[1, TCH]],
                    channel_multiplier=-1,
                )
                masks.append(m)

            # ---- weights (fp16, resident) ----
            wq_src = wq.rearrange("h (c p) m -> p h c m", p=H)
            wqs = []
            for i in range(NHC):
                wq_h = wpool.tile([H, KPC, H], f16, tag=f"wq{i}",
                                  name=f"wq{i}")
                for c8 in range(4):
                    sl = slice(c8 * 8, (c8 + 1) * 8)
                    nc.sync.dma_start(out=wq_h[:, sl, :],
                                      in_=wq_src[:, i, sl, :])
                wqs.append(wq_h)
            wk_sb = wpool.tile([H, KPC, H], f16, tag="wk")
            wk_src = wk.rearrange("(c p) m -> p c m", p=H)
            wv_sb = wpool.tile([H, KPC, H], f16, tag="wv")
            wv_src = wv.rearrange("(c p) m -> p c m", p=H)
            for c16 in range(2):
                sl = slice(c16 * 16, (c16 + 1) * 16)
                nc.sync.dma_start(out=wk_sb[:, sl, :], in_=wk_src[:, sl, :])
                nc.sync.dma_start(out=wv_sb[:, sl, :], in_=wv_src[:, sl, :])
            wo_sb = wpool.tile([H, NHC, D], f16)
            wo_src = wo.rearrange("h p d -> p h d")
            cos_sb = [wpool.tile([H, T], f16, tag=f"cos{b}", name=f"cos{b}")
                      for b in range(B)]
            sin_sb = [wpool.tile([H, T], f16, tag=f"sin{b}", name=f"sin{b}")
                      for b in range(B)]

            def load_cold_weights():
                # wo and the rope tables are first needed ~40us into the
                # kernel; loading them up front put 6MB of DMA ahead of the
                # first x tiles and stalled the PE 48us at startup.
                for bb in range(B):
                    nc.sync.dma_start(out=cos_sb[bb],
                                      in_=cosT[:, bb * T:(bb + 1) * T])
                    nc.sync.dma_start(out=sin_sb[bb],
                                      in_=sinT[:, bb * T:(bb + 1) * T])
                for dc8 in range(8):
                    sl = slice(dc8 * TCH, (dc8 + 1) * TCH)
                    nc.sync.dma_start(out=wo_sb[:, :, sl],
                                      in_=wo_src[:, :, sl])

            # ---- per-batch activations (fp16, both batches resident) ----
            qTs = [[acts.tile([H, NHC, TCH], f16, tag=f"qT{b}_{i}",
                              name=f"qT{b}_{i}") for i in range(NTCH)]
                   for b in range(B)]
            kTs = [[acts.tile([H, TCH], f16, tag=f"kT{b}_{i}",
                              name=f"kT{b}_{i}") for i in range(NTCH)]
                   for b in range(B)]
            vs = [[acts.tile([H, NSUB, H], f16, tag=f"v{b}_{i}",
                             name=f"v{b}_{i}") for i in range(NTCH)]
                  for b in range(B)]

            # ---- PSUM banks (static): two double-banks + four singles ----
            b01 = pbank.tile([H, 2 * TCH], f32, tag="b01", name="b01")
            b23 = pbank.tile([H, 2 * TCH], f32, tag="b23", name="b23")
            bank1 = [pbank.tile([H, TCH], f32, tag=f"b{i}", name=f"bank{i}")
                     for i in range(4, 8)]
            # proj accumulation targets q0..q3, k, v:
            qslices = [b01[:, 0:TCH], b01[:, TCH:2 * TCH],
                       b23[:, 0:TCH], b23[:, TCH:2 * TCH]]
            kbank, vbank = bank1[0], bank1[1]
            avbanks = [bank1[0], bank1[1]]
            sbanks = [bank1[2], bank1[3]]
            # o-projection rotation over the four 512-wide half-banks
            obanks = [b01[:, 0:TCH], b23[:, 0:TCH],
                      b01[:, TCH:2 * TCH], b23[:, TCH:2 * TCH]]

            # ================= emission helpers =================

            def proj(b, c):
                """q0..q3,k,v of chunk c in one x sweep (6 groups)."""
                t0 = b * T + c * TCH
                for k in range(KPC):
                    x_t = xpool.tile([H, TCH], f16, tag="x", bufs=8,
                                     name="xt")
                    nc.sync.dma_start(
                        out=x_t, in_=xT[k * H:(k + 1) * H, t0:t0 + TCH])
                    st, sp = (k == 0), (k == KPC - 1)
                    for i in range(NHC):
                        nc.tensor.matmul(qslices[i], wqs[i][:, k, :], x_t,
                                         start=st, stop=sp,
                                         skip_group_check=True)
                    nc.tensor.matmul(kbank, wk_sb[:, k, :], x_t,
                                     start=st, stop=sp, skip_group_check=True)
                    nc.tensor.matmul(vbank, wv_sb[:, k, :], x_t,
                                     start=st, stop=sp, skip_group_check=True)

            def make_rope_units(b, c):
                """rels: ACT bank releases ordered by when attention needs
                the bank (b5 av/h1, b4 av/h0, then q banks for oproj).
                maths: DVE rope math per group. vtp: PE v-transpose."""
                cs = cos_sb[b][:, c * TCH:(c + 1) * TCH]
                sn = sin_sb[b][:, c * TCH:(c + 1) * TCH]
                staged = {}

                def rel(g, src):
                    # DVE does the bank-release copies: they sit at the
                    # attention block start where the DVE queue is light,
                    # keeping ACT free for exps + o-evictions.
                    def f():
                        d = rope.tile([H, TCH], f16, tag="dir", bufs=6,
                                      name="direct")
                        nc.vector.tensor_copy(d, src)
                        staged[g] = d
                    return f

                def math(g, dst_first, dst_second):
                    def f():
                        d = staged[g]
                        sw = rope.tile([H, TCH], f16, tag="swp", bufs=4,
                                       name="swap")
                        nc.vector.tensor_copy(sw[0:HH, :], d[HH:H, :])
                        nc.vector.tensor_copy(sw[HH:H, :], d[0:HH, :])
                        t1 = rope.tile([H, TCH], f16, tag="t1", bufs=3,
                                       name="t1")
                        t2 = rope.tile([H, TCH], f16, tag="t2", bufs=3,
                                       name="t2")
                        nc.vector.tensor_mul(t1, sw, sn)
                        nc.vector.tensor_mul(t2, d, cs)
                        nc.vector.tensor_sub(dst_first, t2[0:HH, :],
                                             t1[0:HH, :])
                        nc.vector.tensor_add(dst_second, t2[HH:H, :],
                                             t1[HH:H, :])
                    return f

                vst = {}

                def vstage_rel():
                    v = rope.tile([H, TCH], f16, tag="vs", bufs=2,
                                  name="vstage")
                    nc.scalar.activation(v, vbank, Copy)
                    vst["t"] = v

                def vtp():
                    # v transpose via the DMA XBAR (2-byte transpose mode):
                    # no PSUM bank, no PE, no DVE evicts.
                    v = vst["t"]
                    for j in range(NSUB):
                        nc.sync.dma_start_transpose(
                            vs[b][c][:, j, :], v[:, j * H:(j + 1) * H])

                rels = [vstage_rel, rel("k", kbank)]
                qrels = [rel("q0", qslices[0]), rel("q1", qslices[1]),
                         rel("q2", qslices[2]), rel("q3", qslices[3])]
                maths = [math("k", kTs[b][c][0:HH, :], kTs[b][c][HH:H, :])]
                for i in range(NHC):
                    maths.append(math(f"q{i}", qTs[b][c][0:HH, i, :],
                                      qTs[b][c][HH:H, i, :]))
                return rels, qrels, maths, vtp

            def oproj_u(pend, u):
                """One t-subtile (u) of the pending block's o-projection.
                Groups rotate over the four half-banks; each full double-bank
                (two groups) is evicted with ONE [128,1024] ACT copy."""
                pb, pqc, outT = pend
                trow = pb * T + pqc * TCH + u * H
                for dc in range(D // TCH):
                    ob = obanks[dc % 4]
                    for hh in range(NHC):
                        nc.tensor.matmul(
                            ob, outT[:, hh, u * H:(u + 1) * H],
                            wo_sb[:, hh, dc * TCH:(dc + 1) * TCH],
                            start=(hh == 0), stop=(hh == NHC - 1),
                            skip_group_check=True)
                    o_sb = opool.tile([H, TCH], f16, tag="osb",
                                      bufs=6, name="osb")
                    if u == NSUB - 1:
                        # block-end evictions go to DVE so the ACT queue is
                        # clear for the next block's first exps
                        nc.vector.tensor_copy(o_sb, ob)
                    else:
                        nc.scalar.activation(o_sb, ob, Copy)
                    nc.sync.dma_start(
                        out=o_part[trow:trow + H, dc * TCH:(dc + 1) * TCH],
                        in_=o_sb)

            def attn_head(b, qc, h, outT, rels):
                """scores+av+denominator+normalize for one head; rels are
                interleaved ACT bank releases popped after the first score
                blocks."""
                n_st = (qc + 1) * NSUB
                rhs_q = qTs[b][qc][:, h, :]
                av_bank = avbanks[h % 2]
                p2ring = {}
                lsum = [None]

                def scores(st):
                    sps = sbanks[st % 2]
                    kt = kTs[b][st // NSUB][:, (st % NSUB) * H:
                                            (st % NSUB + 1) * H]
                    j = st - qc * NSUB
                    if j >= 0:
                        # diagonal block: seed the accumulation with the
                        # additive causal mask (I.T @ maskneg = maskneg)
                        nc.tensor.matmul(sps, identity, masks[j],
                                         start=True, stop=False,
                                         skip_group_check=True)
                    nc.tensor.matmul(sps, kt, rhs_q, start=(j < 0),
                                     stop=True, skip_group_check=True)
                    pT2 = ppool.tile([H, TCH], f16, tag="p2", bufs=DEPTH + 2,
                                     name="pT2")
                    nc.scalar.activation(pT2, sps, Exp, scale=C_SM)
                    p2ring[st] = pT2

                def avl(st):
                    pT2 = p2ring.pop(st)
                    nc.tensor.matmul(av_bank,
                                     vs[b][st // NSUB][:, st % NSUB, :],
                                     pT2, start=(st == 0),
                                     stop=(st == n_st - 1),
                                     skip_group_check=True)
                    # fp16 running sum of exp'd blocks (softmax denominator);
                    # fp16 is deliberate: ~0.1% on l -> ~0.1% output scale,
                    # well inside the error budget, and it keeps DVE at 2x.
                    nl = ppool.tile([H, TCH], f16, tag="ls", bufs=2,
                                    name="lsum")
                    if lsum[0] is None:
                        nc.vector.tensor_copy(nl, pT2)
                    else:
                        with nc.allow_low_precision(reason="fp16 lsum"):
                            nc.vector.tensor_add(nl, lsum[0], pT2)
                    lsum[0] = nl

                for st in range(n_st):
                    scores(st)
                    if rels:
                        rels.pop(0)()
                    if st >= DEPTH:
                        avl(st - DEPTH)
                while rels:
                    rels.pop(0)()
                for st in range(max(0, n_st - DEPTH), n_st):
                    avl(st)

                # free the av bank immediately (DVE copy) so the next proj's
                # k/v groups never wait on the denominator chain, which is
                # slow (partition_all_reduce ~3us on GpSimd): partition-sum+
                # broadcast of lsum, fp16 reciprocal, then fold 1/l into the
                # raw attention output at DVE 2x rate.
                avraw = small.tile([H, TCH], f16, tag="ar", bufs=2,
                                   name="avraw")
                nc.vector.tensor_copy(avraw, av_bank)
                lbc = small.tile([H, TCH], f16, tag="lb", bufs=2, name="lbc")
                nc.gpsimd.partition_all_reduce(lbc, lsum[0], channels=H,
                                               reduce_op=bass_isa.ReduceOp.add)
                rl = small.tile([H, TCH], f16, tag="rl", bufs=2, name="rl")
                with nc.allow_low_precision(reason="fp16 softmax recip"):
                    nc.vector.reciprocal(rl, lbc)
                nc.vector.tensor_mul(outT[:, h, :], avraw, rl)

            # ================= schedule =================

            pending = None   # (b, qc, outT) awaiting o-projection
            slot = None      # (b, qc) attention block to emit next
            for b in range(B):
                for c in range(NTCH):
                    proj(b, c)
                    if b == 0 and c == 0:
                        load_cold_weights()
                    rels, qrels, maths, vtp = make_rope_units(b, c)
                    if slot is None:
                        for r in rels + qrels:
                            r()
                        for m in maths:
                            m()
                        vtp()
                    else:
                        sb, sqc = slot
                        outT = otpool.tile([H, NHC, TCH], f16, tag="outT",
                                           bufs=2, name="outT")
                        # rope math per head: 2,2,1,0 so the DVE queue is
                        # clear of rope work well before the block ends (the
                        # next proj chunk's first banks depend on DVE-queued
                        # releases otherwise)
                        nmath = [2, 2, 1, 0]
                        for h in range(NHC):
                            attn_head(sb, sqc, h, outT,
                                      rels if h == 0 else [])
                            if h == 0:
                                vtp()
                                # q-bank releases: needed by the oproj
                                # below, kept out of the block-start DVE queue
                                for r in qrels:
                                    r()
                            for _ in range(nmath[h]):
                                if maths:
                                    maths.pop(0)()
                            if pending is not None:
                                oproj_u(pending, h)
                        pending = (sb, sqc, outT)
                    slot = (b, c)

            # tail: attention for the last chunk, then its o-projection
            sb, sqc = slot
            outT = otpool.tile([H, NHC, TCH], f16, tag="outT", bufs=2,
                               name="outT")
            for h in range(NHC):
                attn_head(sb, sqc, h, outT, [])
                if pending is not None:
                    oproj_u(pending, h)
            pending = (sb, sqc, outT)
            for u in range(NSUB):
                oproj_u(pending, u)

    nc.compile()
    return nc


_NC_CACHE = None


def kernel(x, wq, wk, wv, wo, positions):
    global _NC_CACHE
    from concourse.bass_utils import run_bass_kernel_spmd

    x = np.asarray(x, dtype=np.float32)
    wq = np.asarray(wq, dtype=np.float32)
    wk = np.asarray(wk, dtype=np.float32)
    wv = np.asarray(wv, dtype=np.float32)
    wo = np.asarray(wo, dtype=np.float32)
    positions = np.asarray(positions)

    xT = np.ascontiguousarray(x.reshape(TOK, D).T.astype(np.float16))
    # rope tables, [H/2, B*T], duplicated across both partition halves
    fraction = 2.0 * np.arange(HH, dtype=np.float32) / H
    timescale = (THETA ** fraction).astype(np.float32)
    pos = positions.reshape(TOK).astype(np.float32)
    sinusoid = pos[None, :] / timescale[:, None]
    cosT = np.cos(sinusoid).astype(np.float16)
    sinT = np.sin(sinusoid).astype(np.float16)
    cosT = np.ascontiguousarray(np.concatenate([cosT, cosT], axis=0))
    sinT = np.ascontiguousarray(np.concatenate([sinT, sinT], axis=0))

    wq16 = wq.astype(np.float16)
    wk16 = wk.astype(np.float16)
    wv16 = wv.astype(np.float16)
    wo16 = wo.astype(np.float16)

    if _NC_CACHE is None:
        _NC_CACHE = _build_bass()
    nc = _NC_CACHE

    in_maps = []
    for c in range(NCORES):
        in_maps.append({
            "xT": xT,
            "wq": np.ascontiguousarray(wq16[c * NHC:(c + 1) * NHC]),
            "wk": np.ascontiguousarray(wk16[c]),
            "wv": np.ascontiguousarray(wv16[c]),
            "wo": np.ascontiguousarray(wo16[c * NHC:(c + 1) * NHC]),
            "cosT": cosT,
            "sinT": sinT,
        })

    trace = os.environ.get("BASS_KERNEL_TRACE", "0") == "1"
    res = run_bass_kernel_spmd(nc, in_maps, list(range(NCORES)), trace=trace)
    global LAST_RESULTS
    LAST_RESULTS = res
    out = np.zeros((TOK, D), dtype=np.float32)
    for c in range(NCORES):
        out += res.results[c]["o_part"].astype(np.float32)
    return out.reshape(B, T, D)


LAST_RESULTS = None


# revision 31
# speedup vs baseline: 1.0694x; 1.0694x over previous
"""GQA causal-attention prefill kernel for Trainium2, tensor-parallel over 8
NeuronCores.  v3: fp16 operands, static PSUM banks, interleaved schedule,
matmul-free softmax denominator.

Reference semantics: q/k/v projections + RoPE + causal GQA attention + output
projection, fp32, B=2, T=2048, D=4096, 32 q heads, 8 kv heads, head_dim 128.

Sharding: head-parallel. Core c gets q heads [4c, 4c+4), kv head c, and the
matching wo slice; each core computes a full-shape partial output
o_part = attn(heads of c) @ wo_c (fp16) and the host sums the 8 partials in
fp32.

Design (v1 baseline 1.29ms -> v2 1.15ms -> v3):
  - All matmul operands fp16: full PE rate at any free width, DVE 2-byte 2x
    mode, half DMA, no f32r cast penalties. PSUM f32. rel err ~5e-4.
  - Single TileContext-lifetime pools: SBUF addresses never migrate between
    phases (v1 lost 22us/batch to cross-phase SBUF reuse stalls).
  - PSUM banks statically time-shared:
      b01,b23 ([128,1024] = 2 banks each): proj q0..q3 halves, then the
        o-projection rotates its accumulation groups over the four 512-wide
        halves (4-deep rotation hides the evictions).
      b4,b5: proj k,v, then av accumulation (alternating heads)
      b6,b7: scores double-buffer (+ v-transpose staging on b7)
  - Interleaved schedule: proj(chunk c) -> attention(q-chunk c-1) with the
    o-projection of the block before that woven in per head. The Tile
    scheduler then overlaps everything; the PE stream has no phase cliffs
    and stays in the 2.4GHz p-state (idle gaps drop it to 1.2GHz).
  - softmax denominator without matmuls: DVE keeps a running fp16 sum of the
    exp'd score blocks (2x mode, off the critical path), GpSimd
    partition_all_reduce turns it into the broadcast denominator, DVE
    reciprocal + multiply fold 1/l into the attention output. This removes
    320 PE matmuls (~98us) the v1/v2 kernels spent on ones-column matmuls.
  - exp on ACT writes pT2 fp16 directly for off-diagonal blocks; diagonal
    blocks get a 0/1 wedge-mask multiply on DVE.
  - o-projection evictions alternate ACT/DVE so neither engine's queue gates
    the PE's o-matmul bank rotation.
"""

import os
import sys

sys.path.insert(0, "/opt/trn_rl_repo")

import numpy as np

B = 2
T = 2048
TOK = B * T
D = 4096
NQ = 32
NKV = 8
H = 128
HH = H // 2
THETA = 10000.0
NCORES = 8
NHC = NQ // NCORES          # q heads per core (4)
KPC = D // H                # contraction chunks of 128 over D (32)
TCH = 512                   # token chunk
NTCH = T // TCH             # 4 token chunks per batch
NSUB = TCH // H             # 4 128-sub-tiles per chunk
C_SM = 1.0 / np.sqrt(H)     # softmax scale
DEPTH = 3                   # score->av software pipeline depth


def _build_bass():
    import concourse.bacc as bacc
    import concourse.mybir as mybir
    import concourse.tile as tile
    from concourse import bass_isa
    from concourse.masks import make_identity
    from contextlib import ExitStack

    f16 = mybir.dt.float16
    f32 = mybir.dt.float32
    Exp = mybir.ActivationFunctionType.Exp
    Copy = mybir.ActivationFunctionType.Copy

    nc = bacc.Bacc("TRN2", target_bir_lowering=False, debug=False,
                   num_devices=NCORES)

    xT = nc.declare_dram_parameter("xT", [D, TOK], f16, isOutput=False)
    wq = nc.declare_dram_parameter("wq", [NHC, D, H], f16, isOutput=False)
    wk = nc.declare_dram_parameter("wk", [D, H], f16, isOutput=False)
    wv = nc.declare_dram_parameter("wv", [D, H], f16, isOutput=False)
    wo = nc.declare_dram_parameter("wo", [NHC, H, D], f16, isOutput=False)
    # rope tables with both partition halves duplicated (row p == row p+64)
    cosT = nc.declare_dram_parameter("cosT", [H, TOK], f16, isOutput=False)
    sinT = nc.declare_dram_parameter("sinT", [H, TOK], f16, isOutput=False)
    o_part = nc.declare_dram_parameter("o_part", [TOK, D], f16, isOutput=True)

    with tile.TileContext(nc) as tc:
        with ExitStack() as top:
            consts = top.enter_context(tc.tile_pool(name="consts", bufs=1))
            wpool = top.enter_context(tc.tile_pool(name="wpool", bufs=1))
            acts = top.enter_context(tc.tile_pool(name="acts", bufs=1))
            xpool = top.enter_context(tc.tile_pool(name="xpool", bufs=1))
            rope = top.enter_context(tc.tile_pool(name="rope", bufs=1))
            ppool = top.enter_context(tc.tile_pool(name="ppool", bufs=1))
            small = top.enter_context(tc.tile_pool(name="small", bufs=1))
            otpool = top.enter_context(tc.tile_pool(name="otpool", bufs=1))
            opool = top.enter_context(tc.tile_pool(name="opool", bufs=1))
            pbank = top.enter_context(
                tc.tile_pool(name="pbank", bufs=1, space="PSUM"))

            # ---- constants ----
            # fp16 identity: stationary operand for the additive-mask matmul
            identity = consts.tile([H, H], f16)
            make_identity(nc, identity)
            # additive causal wedge masks: 0 where t - s - 128*j >= 0
            # (allowed), -30000 where masked; accumulated into the scores
            # PSUM via I.T @ maskneg so the causal mask costs one PE matmul
            # instead of a DVE multiply on the exp->av critical path.
            masks = []
            for j in range(NSUB):
                m = consts.tile([H, TCH], f16, tag=f"mask{j}",
                                name=f"mask{j}")
                nc.vector.memset(m, 0.0)
                nc.gpsimd.affine_select(
                    out=m, in_=m,
                    compare_op=mybir.AluOpType.is_ge,
                    fill=-30000.0,
                    base=-H * j,
                    pattern=[[1, TCH]],
                    channel_multiplier=-1,
                )
                masks.append(m)

            # ---- weights (fp16, resident) ----
            wq_src = wq.rearrange("h (c p) m -> p h c m", p=H)
            wqs = []
            for i in range(NHC):
                wq_h = wpool.tile([H, KPC, H], f16, tag=f"wq{i}",
                                  name=f"wq{i}")
                for c8 in range(4):
                    sl = slice(c8 * 8, (c8 + 1) * 8)
                    nc.sync.dma_start(out=wq_h[:, sl, :],
                                      in_=wq_src[:, i, sl, :])
                wqs.append(wq_h)
            wk_sb = wpool.tile([H, KPC, H], f16, tag="wk")
            wk_src = wk.rearrange("(c p) m -> p c m", p=H)
            wv_sb = wpool.tile([H, KPC, H], f16, tag="wv")
            wv_src = wv.rearrange("(c p) m -> p c m", p=H)
            for c16 in range(2):
                sl = slice(c16 * 16, (c16 + 1) * 16)
                nc.sync.dma_start(out=wk_sb[:, sl, :], in_=wk_src[:, sl, :])
                nc.sync.dma_start(out=wv_sb[:, sl, :], in_=wv_src[:, sl, :])
            wo_sb = wpool.tile([H, NHC, D], f16)
            wo_src = wo.rearrange("h p d -> p h d")
            cos_sb = [wpool.tile([H, T], f16, tag=f"cos{b}", name=f"cos{b}")
                      for b in range(B)]
            sin_sb = [wpool.tile([H, T], f16, tag=f"sin{b}", name=f"sin{b}")
                      for b in range(B)]

            def load_cold_weights():
                # wo and the rope tables are first needed ~40us into the
                # kernel; loading them up front put 6MB of DMA ahead of the
                # first x tiles and stalled the PE ~48us at startup.
                for bb in range(B):
                    nc.sync.dma_start(out=cos_sb[bb],
                                      in_=cosT[:, bb * T:(bb + 1) * T])
                    nc.sync.dma_start(out=sin_sb[bb],
                                      in_=sinT[:, bb * T:(bb + 1) * T])
                for dc8 in range(8):
                    sl = slice(dc8 * TCH, (dc8 + 1) * TCH)
                    nc.sync.dma_start(out=wo_sb[:, :, sl],
                                      in_=wo_src[:, :, sl])

            # ---- per-batch activations (fp16, both batches resident) ----
            qTs = [[acts.tile([H, NHC, TCH], f16, tag=f"qT{b}_{i}",
                              name=f"qT{b}_{i}") for i in range(NTCH)]
                   for b in range(B)]
            kTs = [[acts.tile([H, TCH], f16, tag=f"kT{b}_{i}",
                              name=f"kT{b}_{i}") for i in range(NTCH)]
                   for b in range(B)]
            vs = [[acts.tile([H, NSUB, H], f16, tag=f"v{b}_{i}",
                             name=f"v{b}_{i}") for i in range(NTCH)]
                  for b in range(B)]

            # ---- PSUM banks (static): two double-banks + four singles ----
            b01 = pbank.tile([H, 2 * TCH], f32, tag="b01", name="b01")
            b23 = pbank.tile([H, 2 * TCH], f32, tag="b23", name="b23")
            bank1 = [pbank.tile([H, TCH], f32, tag=f"b{i}", name=f"bank{i}")
                     for i in range(4, 8)]
            # proj accumulation targets q0..q3, k, v:
            qslices = [b01[:, 0:TCH], b01[:, TCH:2 * TCH],
                       b23[:, 0:TCH], b23[:, TCH:2 * TCH]]
            kbank, vbank = bank1[0], bank1[1]
            avbanks = [bank1[0], bank1[1]]
            sbanks = [bank1[2], bank1[3]]
            # o-projection rotation over the four 512-wide half-banks
            obanks = [b01[:, 0:TCH], b23[:, 0:TCH],
                      b01[:, TCH:2 * TCH], b23[:, TCH:2 * TCH]]

            # ================= emission helpers =================

            def proj(b, c):
                """q0..q3,k,v of chunk c in one x sweep (6 groups)."""
                t0 = b * T + c * TCH
                for k in range(KPC):
                    x_t = xpool.tile([H, TCH], f16, tag="x", bufs=8,
                                     name="xt")
                    nc.sync.dma_start(
                        out=x_t, in_=xT[k * H:(k + 1) * H, t0:t0 + TCH])
                    st, sp = (k == 0), (k == KPC - 1)
                    for i in range(NHC):
                        nc.tensor.matmul(qslices[i], wqs[i][:, k, :], x_t,
                                         start=st, stop=sp,
                                         skip_group_check=True)
                    nc.tensor.matmul(kbank, wk_sb[:, k, :], x_t,
                                     start=st, stop=sp, skip_group_check=True)
                    nc.tensor.matmul(vbank, wv_sb[:, k, :], x_t,
                                     start=st, stop=sp, skip_group_check=True)

            def make_rope_units(b, c):
                """rels: ACT bank releases ordered by when attention needs
                the bank (b5 av/h1, b4 av/h0, then q banks for oproj).
                maths: DVE rope math per group. vtp: PE v-transpose."""
                cs = cos_sb[b][:, c * TCH:(c + 1) * TCH]
                sn = sin_sb[b][:, c * TCH:(c + 1) * TCH]
                staged = {}

                def rel(g, src):
                    # DVE does the bank-release copies: they sit at the
                    # attention block start where the DVE queue is light,
                    # keeping ACT free for exps + o-evictions.
                    def f():
                        d = rope.tile([H, TCH], f16, tag="dir", bufs=6,
                                      name="direct")
                        nc.vector.tensor_copy(d, src)
                        staged[g] = d
                    return f

                def math(g, dst_first, dst_second):
                    def f():
                        d = staged[g]
                        sw = rope.tile([H, TCH], f16, tag="swp", bufs=4,
                                       name="swap")
                        nc.vector.tensor_copy(sw[0:HH, :], d[HH:H, :])
                        nc.vector.tensor_copy(sw[HH:H, :], d[0:HH, :])
                        t1 = rope.tile([H, TCH], f16, tag="t1", bufs=3,
                                       name="t1")
                        t2 = rope.tile([H, TCH], f16, tag="t2", bufs=3,
                                       name="t2")
                        nc.vector.tensor_mul(t1, sw, sn)
                        nc.vector.tensor_mul(t2, d, cs)
                        nc.vector.tensor_sub(dst_first, t2[0:HH, :],
                                             t1[0:HH, :])
                        nc.vector.tensor_add(dst_second, t2[HH:H, :],
                                             t1[HH:H, :])
                    return f

                vst = {}

                def vstage_rel():
                    v = rope.tile([H, TCH], f16, tag="vs", bufs=2,
                                  name="vstage")
                    nc.scalar.activation(v, vbank, Copy)
                    vst["t"] = v

                def vtp():
                    # v transpose via the DMA XBAR (2-byte transpose mode):
                    # no PSUM bank, no PE, no DVE evicts.
                    v = vst["t"]
                    for j in range(NSUB):
                        nc.sync.dma_start_transpose(
                            vs[b][c][:, j, :], v[:, j * H:(j + 1) * H])

                rels = [vstage_rel, rel("k", kbank)]
                qrels = [rel("q0", qslices[0]), rel("q1", qslices[1]),
                         rel("q2", qslices[2]), rel("q3", qslices[3])]
                maths = [math("k", kTs[b][c][0:HH, :], kTs[b][c][HH:H, :])]
                for i in range(NHC):
                    maths.append(math(f"q{i}", qTs[b][c][0:HH, i, :],
                                      qTs[b][c][HH:H, i, :]))
                return rels, qrels, maths, vtp

            def oproj_u(pend, u):
                """One t-subtile (u) of the pending block's o-projection.
                Groups rotate over the four half-banks; each full double-bank
                (two groups) is evicted with ONE [128,1024] ACT copy."""
                pb, pqc, outT = pend
                trow = pb * T + pqc * TCH + u * H
                for dc in range(D // TCH):
                    ob = obanks[dc % 4]
                    for hh in range(NHC):
                        nc.tensor.matmul(
                            ob, outT[:, hh, u * H:(u + 1) * H],
                            wo_sb[:, hh, dc * TCH:(dc + 1) * TCH],
                            start=(hh == 0), stop=(hh == NHC - 1),
                            skip_group_check=True)
                    o_sb = opool.tile([H, TCH], f16, tag="osb",
                                      bufs=6, name="osb")
                    nc.scalar.activation(o_sb, ob, Copy)
                    nc.sync.dma_start(
                        out=o_part[trow:trow + H, dc * TCH:(dc + 1) * TCH],
                        in_=o_sb)

            def attn_head(b, qc, h, outT, rels):
                """scores+av+denominator+normalize for one head; rels are
                interleaved ACT bank releases popped after the first score
                blocks."""
                n_st = (qc + 1) * NSUB
                rhs_q = qTs[b][qc][:, h, :]
                av_bank = avbanks[h % 2]
                p2ring = {}
                lsum = [None]

                def scores(st):
                    sps = sbanks[st % 2]
                    kt = kTs[b][st // NSUB][:, (st % NSUB) * H:
                                            (st % NSUB + 1) * H]
                    j = st - qc * NSUB
                    if j >= 0:
                        # diagonal block: seed the accumulation with the
                        # additive causal mask (I.T @ maskneg = maskneg)
                        nc.tensor.matmul(sps, identity, masks[j],
                                         start=True, stop=False,
                                         skip_group_check=True)
                    nc.tensor.matmul(sps, kt, rhs_q, start=(j < 0),
                                     stop=True, skip_group_check=True)
                    pT2 = ppool.tile([H, TCH], f16, tag="p2", bufs=DEPTH + 2,
                                     name="pT2")
                    nc.scalar.activation(pT2, sps, Exp, scale=C_SM)
                    p2ring[st] = pT2

                def avl(st):
                    pT2 = p2ring.pop(st)
                    nc.tensor.matmul(av_bank,
                                     vs[b][st // NSUB][:, st % NSUB, :],
                                     pT2, start=(st == 0),
                                     stop=(st == n_st - 1),
                                     skip_group_check=True)
                    # fp16 running sum of exp'd blocks (softmax denominator);
                    # fp16 is deliberate: ~0.1% on l -> ~0.1% output scale,
                    # well inside the error budget, and it keeps DVE at 2x.
                    nl = ppool.tile([H, TCH], f16, tag="ls", bufs=2,
                                    name="lsum")
                    if lsum[0] is None:
                        nc.vector.tensor_copy(nl, pT2)
                    else:
                        with nc.allow_low_precision(reason="fp16 lsum"):
                            nc.vector.tensor_add(nl, lsum[0], pT2)
                    lsum[0] = nl

                for st in range(n_st):
                    scores(st)
                    if rels:
                        rels.pop(0)()
                    if st >= DEPTH:
                        avl(st - DEPTH)
                while rels:
                    rels.pop(0)()
                for st in range(max(0, n_st - DEPTH), n_st):
                    avl(st)

                # free the av bank immediately (DVE copy) so the next proj's
                # k/v groups never wait on the denominator chain, which is
                # slow (partition_all_reduce ~3us on GpSimd): partition-sum+
                # broadcast of lsum, fp16 reciprocal, then fold 1/l into the
                # raw attention output at DVE 2x rate.
                avraw = small.tile([H, TCH], f16, tag="ar", bufs=2,
                                   name="avraw")
                nc.vector.tensor_copy(avraw, av_bank)
                lbc = small.tile([H, TCH], f16, tag="lb", bufs=2, name="lbc")
                nc.gpsimd.partition_all_reduce(lbc, lsum[0], channels=H,
                                               reduce_op=bass_isa.ReduceOp.add)
                rl = small.tile([H, TCH], f16, tag="rl", bufs=2, name="rl")
                with nc.allow_low_precision(reason="fp16 softmax recip"):
                    nc.vector.reciprocal(rl, lbc)
                nc.vector.tensor_mul(outT[:, h, :], avraw, rl)

            # ================= schedule =================

            pending = None   # (b, qc, outT) awaiting o-projection
            slot = None      # (b, qc) attention block to emit next
            for b in range(B):
                for c in range(NTCH):
                    proj(b, c)
                    if b == 0 and c == 0:
                        load_cold_weights()
                    rels, qrels, maths, vtp = make_rope_units(b, c)
                    if slot is None:
                        for r in rels + qrels:
                            r()
                        for m in maths:
                            m()
                        vtp()
                    else:
                        sb, sqc = slot
                        outT = otpool.tile([H, NHC, TCH], f16, tag="outT",
                                           bufs=2, name="outT")
                        # rope math per head: 2,2,1,0 so the DVE queue is
                        # clear of rope work well before the block ends (the
                        # next proj chunk's first banks depend on DVE-queued
                        # releases otherwise)
                        nmath = [2, 2, 1, 0]
                        for h in range(NHC):
                            attn_head(sb, sqc, h, outT,
                                      rels if h == 0 else [])
                            if h == 0:
                                vtp()
                                # q-bank releases: needed by the oproj
                                # below, kept out of the block-start DVE queue
                                for r in qrels:
                                    r()
                            for _ in range(nmath[h]):
                                if maths:
                                    maths.pop(0)()
                            if pending is not None:
                                oproj_u(pending, h)
                        pending = (sb, sqc, outT)
                    slot = (b, c)

            # tail: attention for the last chunk, then its o-projection
            sb, sqc = slot
            outT = otpool.tile([H, NHC, TCH], f16, tag="outT", bufs=2,
                               name="outT")
            for h in range(NHC):
                attn_head(sb, sqc, h, outT, [])
                if pending is not None:
                    oproj_u(pending, h)
            pending = (sb, sqc, outT)
            for u in range(NSUB):
                oproj_u(pending, u)

    nc.compile()
    return nc


_NC_CACHE = None


def kernel(x, wq, wk, wv, wo, positions):
    global _NC_CACHE
    from concourse.bass_utils import run_bass_kernel_spmd

    x = np.asarray(x, dtype=np.float32)
    wq = np.asarray(wq, dtype=np.float32)
    wk = np.asarray(wk, dtype=np.float32)
    wv = np.asarray(wv, dtype=np.float32)
    wo = np.asarray(wo, dtype=np.float32)
    positions = np.asarray(positions)

    xT = np.ascontiguousarray(x.reshape(TOK, D).T.astype(np.float16))
    # rope tables, [H/2, B*T], duplicated across both partition halves
    fraction = 2.0 * np.arange(HH, dtype=np.float32) / H
    timescale = (THETA ** fraction).astype(np.float32)
    pos = positions.reshape(TOK).astype(np.float32)
    sinusoid = pos[None, :] / timescale[:, None]
    cosT = np.cos(sinusoid).astype(np.float16)
    sinT = np.sin(sinusoid).astype(np.float16)
    cosT = np.ascontiguousarray(np.concatenate([cosT, cosT], axis=0))
    sinT = np.ascontiguousarray(np.concatenate([sinT, sinT], axis=0))

    wq16 = wq.astype(np.float16)
    wk16 = wk.astype(np.float16)
    wv16 = wv.astype(np.float16)
    wo16 = wo.astype(np.float16)

    if _NC_CACHE is None:
        _NC_CACHE = _build_bass()
    nc = _NC_CACHE

    in_maps = []
    for c in range(NCORES):
        in_maps.append({
            "xT": xT,
            "wq": np.ascontiguousarray(wq16[c * NHC:(c + 1) * NHC]),
            "wk": np.ascontiguousarray(wk16[c]),
            "wv": np.ascontiguousarray(wv16[c]),
            "wo": np.ascontiguousarray(wo16[c * NHC:(c + 1) * NHC]),
            "cosT": cosT,
            "sinT": sinT,
        })

    trace = os.environ.get("BASS_KERNEL_TRACE", "0") == "1"
    res = run_bass_kernel_spmd(nc, in_maps, list(range(NCORES)), trace=trace)
    global LAST_RESULTS
    LAST_RESULTS = res
    out = np.zeros((TOK, D), dtype=np.float32)
    for c in range(NCORES):
        out += res.results[c]["o_part"].astype(np.float32)
    return out.reshape(B, T, D)


LAST_RESULTS = None


# revision 33
# speedup vs baseline: 1.2456x; 1.1648x over previous
"""GQA causal-attention prefill kernel for Trainium2, tensor-parallel over 8
NeuronCores.  v3: fp16 operands, static PSUM banks, interleaved schedule,
matmul-free softmax denominator.

Reference semantics: q/k/v projections + RoPE + causal GQA attention + output
projection, fp32, B=2, T=2048, D=4096, 32 q heads, 8 kv heads, head_dim 128.

Sharding: head-parallel. Core c gets q heads [4c, 4c+4), kv head c, and the
matching wo slice; each core computes a full-shape partial output
o_part = attn(heads of c) @ wo_c (fp16) and the host sums the 8 partials in
fp32.

Design (v1 baseline 1.29ms -> v2 1.15ms -> v3):
  - All matmul operands fp16: full PE rate at any free width, DVE 2-byte 2x
    mode, half DMA, no f32r cast penalties. PSUM f32. rel err ~5e-4.
  - Single TileContext-lifetime pools: SBUF addresses never migrate between
    phases (v1 lost 22us/batch to cross-phase SBUF reuse stalls).
  - PSUM banks statically time-shared:
      b01,b23 ([128,1024] = 2 banks each): proj q0..q3 halves, then the
        o-projection rotates its accumulation groups over the four 512-wide
        halves (4-deep rotation hides the evictions).
      b4,b5: proj k,v, then av accumulation (alternating heads)
      b6,b7: scores double-buffer (+ v-transpose staging on b7)
  - Interleaved schedule: proj(chunk c) -> attention(q-chunk c-1) with the
    o-projection of the block before that woven in per head. The Tile
    scheduler then overlaps everything; the PE stream has no phase cliffs
    and stays in the 2.4GHz p-state (idle gaps drop it to 1.2GHz).
  - softmax denominator without matmuls: DVE keeps a running fp16 sum of the
    exp'd score blocks (2x mode, off the critical path), GpSimd
    partition_all_reduce turns it into the broadcast denominator, DVE
    reciprocal + multiply fold 1/l into the attention output. This removes
    320 PE matmuls (~98us) the v1/v2 kernels spent on ones-column matmuls.
  - exp on ACT writes pT2 fp16 directly for off-diagonal blocks; diagonal
    blocks get a 0/1 wedge-mask multiply on DVE.
  - o-projection evictions alternate ACT/DVE so neither engine's queue gates
    the PE's o-matmul bank rotation.
"""

import os
import sys

sys.path.insert(0, "/opt/trn_rl_repo")

import numpy as np

B = 2
T = 2048
TOK = B * T
D = 4096
NQ = 32
NKV = 8
H = 128
HH = H // 2
THETA = 10000.0
NCORES = 8
NHC = NQ // NCORES          # q heads per core (4)
KPC = D // H                # contraction chunks of 128 over D (32)
TCH = 512                   # token chunk
NTCH = T // TCH             # 4 token chunks per batch
NSUB = TCH // H             # 4 128-sub-tiles per chunk
C_SM = 1.0 / np.sqrt(H)     # softmax scale
DEPTH = 3                   # score->av software pipeline depth


def _build_bass():
    import concourse.bacc as bacc
    import concourse.mybir as mybir
    import concourse.tile as tile
    from concourse import bass_isa
    from concourse.masks import make_identity
    from contextlib import ExitStack

    f16 = mybir.dt.float16
    f32 = mybir.dt.float32
    Exp = mybir.ActivationFunctionType.Exp
    Copy = mybir.ActivationFunctionType.Copy

    nc = bacc.Bacc("TRN2", target_bir_lowering=False, debug=False,
                   num_devices=NCORES)

    xT = nc.declare_dram_parameter("xT", [D, TOK], f16, isOutput=False)
    wq = nc.declare_dram_parameter("wq", [NHC, D, H], f16, isOutput=False)
    wk = nc.declare_dram_parameter("wk", [D, H], f16, isOutput=False)
    wv = nc.declare_dram_parameter("wv", [D, H], f16, isOutput=False)
    wo = nc.declare_dram_parameter("wo", [NHC, H, D], f16, isOutput=False)
    # rope tables with both partition halves duplicated (row p == row p+64)
    cosT = nc.declare_dram_parameter("cosT", [H, TOK], f16, isOutput=False)
    sinT = nc.declare_dram_parameter("sinT", [H, TOK], f16, isOutput=False)
    o_part = nc.declare_dram_parameter("o_part", [TOK, D], f16, isOutput=True)

    with tile.TileContext(nc) as tc:
        with ExitStack() as top:
            consts = top.enter_context(tc.tile_pool(name="consts", bufs=1))
            wpool = top.enter_context(tc.tile_pool(name="wpool", bufs=1))
            acts = top.enter_context(tc.tile_pool(name="acts", bufs=1))
            xpool = top.enter_context(tc.tile_pool(name="xpool", bufs=1))
            rope = top.enter_context(tc.tile_pool(name="rope", bufs=1))
            ppool = top.enter_context(tc.tile_pool(name="ppool", bufs=1))
            small = top.enter_context(tc.tile_pool(name="small", bufs=1))
            otpool = top.enter_context(tc.tile_pool(name="otpool", bufs=1))
            opool = top.enter_context(tc.tile_pool(name="opool", bufs=1))
            pbank = top.enter_context(
                tc.tile_pool(name="pbank", bufs=1, space="PSUM"))

            # ---- constants ----
            # fp16 identity: stationary operand for the additive-mask matmul
            identity = consts.tile([H, H], f16)
            make_identity(nc, identity)
            # additive causal wedge masks: 0 where t - s - 128*j >= 0
            # (allowed), -30000 where masked; accumulated into the scores
            # PSUM via I.T @ maskneg so the causal mask costs one PE matmul
            # instead of a DVE multiply on the exp->av critical path.
            masks = []
            for j in range(NSUB):
                m = consts.tile([H, TCH], f16, tag=f"mask{j}",
                                name=f"mask{j}")
                nc.vector.memset(m, 0.0)
                nc.gpsimd.affine_select(
                    out=m, in_=m,
                    compare_op=mybir.AluOpType.is_ge,
                    fill=-30000.0,
                    base=-H * j,
                    pattern=[[1, TCH]],
                    channel_multiplier=-1,
                )
                masks.append(m)

            # ---- weights (fp16, resident) ----
            wq_src = wq.rearrange("h (c p) m -> p h c m", p=H)
            wqs = []
            for i in range(NHC):
                wq_h = wpool.tile([H, KPC, H], f16, tag=f"wq{i}",
                                  name=f"wq{i}")
                for c8 in range(4):
                    sl = slice(c8 * 8, (c8 + 1) * 8)
                    nc.sync.dma_start(out=wq_h[:, sl, :],
                                      in_=wq_src[:, i, sl, :])
                wqs.append(wq_h)
            wk_sb = wpool.tile([H, KPC, H], f16, tag="wk")
            wk_src = wk.rearrange("(c p) m -> p c m", p=H)
            wv_sb = wpool.tile([H, KPC, H], f16, tag="wv")
            wv_src = wv.rearrange("(c p) m -> p c m", p=H)
            for c16 in range(2):
                sl = slice(c16 * 16, (c16 + 1) * 16)
                nc.sync.dma_start(out=wk_sb[:, sl, :], in_=wk_src[:, sl, :])
                nc.sync.dma_start(out=wv_sb[:, sl, :], in_=wv_src[:, sl, :])
            wo_sb = wpool.tile([H, NHC, D], f16)
            wo_src = wo.rearrange("h p d -> p h d")
            for dc8 in range(8):
                sl = slice(dc8 * TCH, (dc8 + 1) * TCH)
                nc.sync.dma_start(out=wo_sb[:, :, sl], in_=wo_src[:, :, sl])
            cos_sb = [wpool.tile([H, T], f16, tag=f"cos{b}", name=f"cos{b}")
                      for b in range(B)]
            sin_sb = [wpool.tile([H, T], f16, tag=f"sin{b}", name=f"sin{b}")
                      for b in range(B)]
            for b in range(B):
                nc.sync.dma_start(out=cos_sb[b], in_=cosT[:, b * T:(b + 1) * T])
                nc.sync.dma_start(out=sin_sb[b], in_=sinT[:, b * T:(b + 1) * T])

            # ---- per-batch activations (fp16, both batches resident) ----
            qTs = [[acts.tile([H, NHC, TCH], f16, tag=f"qT{b}_{i}",
                              name=f"qT{b}_{i}") for i in range(NTCH)]
                   for b in range(B)]
            kTs = [[acts.tile([H, TCH], f16, tag=f"kT{b}_{i}",
                              name=f"kT{b}_{i}") for i in range(NTCH)]
                   for b in range(B)]
            vs = [[acts.tile([H, NSUB, H], f16, tag=f"v{b}_{i}",
                             name=f"v{b}_{i}") for i in range(NTCH)]
                  for b in range(B)]

            # ---- PSUM banks (static): two double-banks + four singles ----
            b01 = pbank.tile([H, 2 * TCH], f32, tag="b01", name="b01")
            b23 = pbank.tile([H, 2 * TCH], f32, tag="b23", name="b23")
            bank1 = [pbank.tile([H, TCH], f32, tag=f"b{i}", name=f"bank{i}")
                     for i in range(4, 8)]
            # proj accumulation targets q0..q3, k, v:
            qslices = [b01[:, 0:TCH], b01[:, TCH:2 * TCH],
                       b23[:, 0:TCH], b23[:, TCH:2 * TCH]]
            kbank, vbank = bank1[0], bank1[1]
            avbanks = [bank1[0], bank1[1]]
            sbanks = [bank1[2], bank1[3]]
            # o-projection rotation over the four 512-wide half-banks
            obanks = [b01[:, 0:TCH], b23[:, 0:TCH],
                      b01[:, TCH:2 * TCH], b23[:, TCH:2 * TCH]]

            # ================= emission helpers =================

            def proj(b, c):
                """q0..q3,k,v of chunk c in one x sweep (6 groups)."""
                t0 = b * T + c * TCH
                for k in range(KPC):
                    x_t = xpool.tile([H, TCH], f16, tag="x", bufs=8,
                                     name="xt")
                    nc.sync.dma_start(
                        out=x_t, in_=xT[k * H:(k + 1) * H, t0:t0 + TCH])
                    st, sp = (k == 0), (k == KPC - 1)
                    for i in range(NHC):
                        nc.tensor.matmul(qslices[i], wqs[i][:, k, :], x_t,
                                         start=st, stop=sp,
                                         skip_group_check=True)
                    nc.tensor.matmul(kbank, wk_sb[:, k, :], x_t,
                                     start=st, stop=sp, skip_group_check=True)
                    nc.tensor.matmul(vbank, wv_sb[:, k, :], x_t,
                                     start=st, stop=sp, skip_group_check=True)

            def make_rope_units(b, c):
                """rels: ACT bank releases ordered by when attention needs
                the bank (b5 av/h1, b4 av/h0, then q banks for oproj).
                maths: DVE rope math per group. vtp: PE v-transpose."""
                cs = cos_sb[b][:, c * TCH:(c + 1) * TCH]
                sn = sin_sb[b][:, c * TCH:(c + 1) * TCH]
                staged = {}

                def rel(g, src):
                    # DVE does the bank-release copies: they sit at the
                    # attention block start where the DVE queue is light,
                    # keeping ACT free for exps + o-evictions.
                    def f():
                        d = rope.tile([H, TCH], f16, tag="dir", bufs=6,
                                      name="direct")
                        nc.vector.tensor_copy(d, src)
                        staged[g] = d
                    return f

                def math(g, dst_first, dst_second):
                    def f():
                        d = staged[g]
                        sw = rope.tile([H, TCH], f16, tag="swp", bufs=4,
                                       name="swap")
                        nc.vector.tensor_copy(sw[0:HH, :], d[HH:H, :])
                        nc.vector.tensor_copy(sw[HH:H, :], d[0:HH, :])
                        t1 = rope.tile([H, TCH], f16, tag="t1", bufs=3,
                                       name="t1")
                        t2 = rope.tile([H, TCH], f16, tag="t2", bufs=3,
                                       name="t2")
                        nc.vector.tensor_mul(t1, sw, sn)
                        nc.vector.tensor_mul(t2, d, cs)
                        nc.vector.tensor_sub(dst_first, t2[0:HH, :],
                                             t1[0:HH, :])
                        nc.vector.tensor_add(dst_second, t2[HH:H, :],
                                             t1[HH:H, :])
                    return f

                vst = {}

                def vstage_rel():
                    v = rope.tile([H, TCH], f16, tag="vs", bufs=2,
                                  name="vstage")
                    nc.scalar.activation(v, vbank, Copy)
                    vst["t"] = v

                def vtp():
                    # v transpose via the DMA XBAR (2-byte transpose mode):
                    # no PSUM bank, no PE, no DVE evicts.
                    v = vst["t"]
                    for j in range(NSUB):
                        nc.sync.dma_start_transpose(
                            vs[b][c][:, j, :], v[:, j * H:(j + 1) * H])

                rels = [vstage_rel, rel("k", kbank)]
                qrels = [rel("q0", qslices[0]), rel("q1", qslices[1]),
                         rel("q2", qslices[2]), rel("q3", qslices[3])]
                maths = [math("k", kTs[b][c][0:HH, :], kTs[b][c][HH:H, :])]
                for i in range(NHC):
                    maths.append(math(f"q{i}", qTs[b][c][0:HH, i, :],
                                      qTs[b][c][HH:H, i, :]))
                return rels, qrels, maths, vtp

            def oproj_u(pend, u):
                """One t-subtile (u) of the pending block's o-projection.
                Groups rotate over the four half-banks; each full double-bank
                (two groups) is evicted with ONE [128,1024] ACT copy."""
                pb, pqc, outT = pend
                trow = pb * T + pqc * TCH + u * H
                for dc in range(D // TCH):
                    ob = obanks[dc % 4]
                    for hh in range(NHC):
                        nc.tensor.matmul(
                            ob, outT[:, hh, u * H:(u + 1) * H],
                            wo_sb[:, hh, dc * TCH:(dc + 1) * TCH],
                            start=(hh == 0), stop=(hh == NHC - 1),
                            skip_group_check=True)
                    # evictions sit slightly later in scheduler priority:
                    # their bank has 4 groups of rotation slack, and at
                    # natural priority a burst of them delays the next
                    # head's exps on ACT (sps then stalls on its bank)
                    with tc.high_priority(offset=-150):
                        o_sb = opool.tile([H, TCH], f16, tag="osb",
                                          bufs=6, name="osb")
                        nc.scalar.activation(o_sb, ob, Copy)
                        nc.sync.dma_start(
                            out=o_part[trow:trow + H,
                                       dc * TCH:(dc + 1) * TCH],
                            in_=o_sb)

            def attn_head(b, qc, h, outT, rels):
                """scores+av+denominator+normalize for one head; rels are
                interleaved ACT bank releases popped after the first score
                blocks."""
                n_st = (qc + 1) * NSUB
                rhs_q = qTs[b][qc][:, h, :]
                av_bank = avbanks[h % 2]
                p2ring = {}
                lsum = [None]

                def scores(st):
                    sps = sbanks[st % 2]
                    kt = kTs[b][st // NSUB][:, (st % NSUB) * H:
                                            (st % NSUB + 1) * H]
                    j = st - qc * NSUB
                    if j >= 0:
                        # diagonal block: seed the accumulation with the
                        # additive causal mask (I.T @ maskneg = maskneg)
                        nc.tensor.matmul(sps, identity, masks[j],
                                         start=True, stop=False,
                                         skip_group_check=True)
                    nc.tensor.matmul(sps, kt, rhs_q, start=(j < 0),
                                     stop=True, skip_group_check=True)
                    pT2 = ppool.tile([H, TCH], f16, tag="p2", bufs=DEPTH + 2,
                                     name="pT2")
                    nc.scalar.activation(pT2, sps, Exp, scale=C_SM)
                    p2ring[st] = pT2

                def avl(st):
                    pT2 = p2ring.pop(st)
                    nc.tensor.matmul(av_bank,
                                     vs[b][st // NSUB][:, st % NSUB, :],
                                     pT2, start=(st == 0),
                                     stop=(st == n_st - 1),
                                     skip_group_check=True)
                    # fp16 running sum of exp'd blocks (softmax denominator);
                    # fp16 is deliberate: ~0.1% on l -> ~0.1% output scale,
                    # well inside the error budget, and it keeps DVE at 2x.
                    nl = ppool.tile([H, TCH], f16, tag="ls", bufs=2,
                                    name="lsum")
                    if lsum[0] is None:
                        nc.vector.tensor_copy(nl, pT2)
                    else:
                        with nc.allow_low_precision(reason="fp16 lsum"):
                            nc.vector.tensor_add(nl, lsum[0], pT2)
                    lsum[0] = nl

                for st in range(n_st):
                    scores(st)
                    if rels:
                        rels.pop(0)()
                    if st >= DEPTH:
                        avl(st - DEPTH)
                while rels:
                    rels.pop(0)()
                for st in range(max(0, n_st - DEPTH), n_st):
                    avl(st)

                # free the av bank immediately (DVE copy) so the next proj's
                # k/v groups never wait on the denominator chain, which is
                # slow (partition_all_reduce ~3us on GpSimd): partition-sum+
                # broadcast of lsum, fp16 reciprocal, then fold 1/l into the
                # raw attention output at DVE 2x rate.
                avraw = small.tile([H, TCH], f16, tag="ar", bufs=2,
                                   name="avraw")
                nc.vector.tensor_copy(avraw, av_bank)
                lbc = small.tile([H, TCH], f16, tag="lb", bufs=2, name="lbc")
                nc.gpsimd.partition_all_reduce(lbc, lsum[0], channels=H,
                                               reduce_op=bass_isa.ReduceOp.add)
                rl = small.tile([H, TCH], f16, tag="rl", bufs=2, name="rl")
                with nc.allow_low_precision(reason="fp16 softmax recip"):
                    nc.vector.reciprocal(rl, lbc)
                nc.vector.tensor_mul(outT[:, h, :], avraw, rl)

            # ================= schedule =================

            pending = None   # (b, qc, outT) awaiting o-projection
            slot = None      # (b, qc) attention block to emit next
            for b in range(B):
                for c in range(NTCH):
                    proj(b, c)
                    rels, qrels, maths, vtp = make_rope_units(b, c)
                    if slot is None:
                        for r in rels + qrels:
                            r()
                        for m in maths:
                            m()
                        vtp()
                    else:
                        sb, sqc = slot
                        outT = otpool.tile([H, NHC, TCH], f16, tag="outT",
                                           bufs=2, name="outT")
                        # rope math per head: 2,2,1,0 so the DVE queue is
                        # clear of rope work well before the block ends (the
                        # next proj chunk's first banks depend on DVE-queued
                        # releases otherwise)
                        nmath = [2, 2, 1, 0]
                        for h in range(NHC):
                            attn_head(sb, sqc, h, outT,
                                      rels if h == 0 else [])
                            if h == 0:
                                vtp()
                                # q-bank releases: needed by the oproj
                                # below, kept out of the block-start DVE queue
                                for r in qrels:
                                    r()
                            for _ in range(nmath[h]):
                                if maths:
                                    maths.pop(0)()
                            if pending is not None:
                                oproj_u(pending, h)
                        pending = (sb, sqc, outT)
                    slot = (b, c)

            # tail: attention for the last chunk, then its o-projection
            sb, sqc = slot
            outT = otpool.tile([H, NHC, TCH], f16, tag="outT", bufs=2,
                               name="outT")
            for h in range(NHC):
                attn_head(sb, sqc, h, outT, [])
                if pending is not None:
                    oproj_u(pending, h)
            pending = (sb, sqc, outT)
            for u in range(NSUB):
                oproj_u(pending, u)

    nc.compile()
    return nc


_NC_CACHE = None


def kernel(x, wq, wk, wv, wo, positions):
    global _NC_CACHE
    from concourse.bass_utils import run_bass_kernel_spmd

    x = np.asarray(x, dtype=np.float32)
    wq = np.asarray(wq, dtype=np.float32)
    wk = np.asarray(wk, dtype=np.float32)
    wv = np.asarray(wv, dtype=np.float32)
    wo = np.asarray(wo, dtype=np.float32)
    positions = np.asarray(positions)

    xT = np.ascontiguousarray(x.reshape(TOK, D).T.astype(np.float16))
    # rope tables, [H/2, B*T], duplicated across both partition halves
    fraction = 2.0 * np.arange(HH, dtype=np.float32) / H
    timescale = (THETA ** fraction).astype(np.float32)
    pos = positions.reshape(TOK).astype(np.float32)
    sinusoid = pos[None, :] / timescale[:, None]
    cosT = np.cos(sinusoid).astype(np.float16)
    sinT = np.sin(sinusoid).astype(np.float16)
    cosT = np.ascontiguousarray(np.concatenate([cosT, cosT], axis=0))
    sinT = np.ascontiguousarray(np.concatenate([sinT, sinT], axis=0))

    wq16 = wq.astype(np.float16)
    wk16 = wk.astype(np.float16)
    wv16 = wv.astype(np.float16)
    wo16 = wo.astype(np.float16)

    if _NC_CACHE is None:
        _NC_CACHE = _build_bass()
    nc = _NC_CACHE

    in_maps = []
    for c in range(NCORES):
        in_maps.append({
            "xT": xT,
            "wq": np.ascontiguousarray(wq16[c * NHC:(c + 1) * NHC]),
            "wk": np.ascontiguousarray(wk16[c]),
            "wv": np.ascontiguousarray(wv16[c]),
            "wo": np.ascontiguousarray(wo16[c * NHC:(c + 1) * NHC]),
            "cosT": cosT,
            "sinT": sinT,
        })

    trace = os.environ.get("BASS_KERNEL_TRACE", "0") == "1"
    res = run_bass_kernel_spmd(nc, in_maps, list(range(NCORES)), trace=trace)
    global LAST_RESULTS
    LAST_RESULTS = res
    out = np.zeros((TOK, D), dtype=np.float32)
    for c in range(NCORES):
        out += res.results[c]["o_part"].astype(np.float32)
    return out.reshape(B, T, D)


LAST_RESULTS = None
